# revision 1
# baseline (speedup 1.0000x reference)
"""Trainium2 Bass kernel for the sparse windowed-attention layer.

Contract: kernel(**inputs) takes the FULL unsharded inputs (as produced by
setup_inputs()) and returns the full (out, attn) pair.  Internally the batch
dim (B=32) is sharded 4-per-core across 8 NeuronCores; projection weights are
replicated.

Key structural facts exploited:
  - The dynamic window mask keeps only columns [la-1, la+3) of the 1024
    score columns alive (W <= 4).  Everything else softmaxes to exactly 0.
  - attn output outside the window is exactly 0; ExternalOutput buffers are
    pre-zeroed by the runtime, so the kernel only writes the live window.
  - softmax without max-subtraction is safe here (|scores| << 88) and lets
    the row-normalization fold into the output-projection epilogue.
"""

import sys
import numpy as np

B, TD, TE, C, E, H = 32, 1024, 1024, 256, 256, 128
WINDOW_BACKWARD, WINDOW_AHEAD = 1, 3
NCORES = 8
BPC = B // NCORES  # batches per core

_CACHE = {}


def _col(ap, n):
    """DRAM vector [n] viewed as a column [n, 1] (partition-major)."""
    import concourse.bass as bass
    return bass.AP(tensor=ap.tensor, offset=ap.offset, ap=[[1, n], [0, 1]])


def _bcast(ap, p, n):
    """DRAM vector [n] broadcast to [p, n] (partition step 0)."""
    import concourse.bass as bass
    return bass.AP(tensor=ap.tensor, offset=ap.offset, ap=[[0, p], [1, n]])


def _build(wlo, whi):
    if "/opt/trn_rl_repo" not in sys.path:
        sys.path.insert(0, "/opt/trn_rl_repo")
    import concourse.bacc as bacc
    import concourse.tile as tile
    from concourse import mybir
    from concourse.masks import make_identity

    W = whi - wlo
    f32 = mybir.dt.float32
    AF = mybir.ActivationFunctionType

    nc = bacc.Bacc(None, target_bir_lowering=False)

    query = nc.dram_tensor("query", [BPC, TD, C], f32, kind="ExternalInput")
    keysw = nc.dram_tensor("keysw", [BPC, E, W], f32, kind="ExternalInput")
    valtw = nc.dram_tensor("valtw", [BPC, E, W], f32, kind="ExternalInput")
    wbias = nc.dram_tensor("wbias", [BPC, W], f32, kind="ExternalInput")
    wqt = nc.dram_tensor("wqt", [H, C], f32, kind="ExternalInput")
    wk = nc.dram_tensor("wk", [E, H], f32, kind="ExternalInput")
    wv = nc.dram_tensor("wv", [E, H], f32, kind="ExternalInput")
    wo = nc.dram_tensor("wo", [H, C], f32, kind="ExternalInput")
    bq = nc.dram_tensor("bq", [H], f32, kind="ExternalInput")
    bk = nc.dram_tensor("bk", [H], f32, kind="ExternalInput")
    bv = nc.dram_tensor("bv", [H], f32, kind="ExternalInput")
    bo = nc.dram_tensor("bo", [C], f32, kind="ExternalInput")
    out = nc.dram_tensor("out", [BPC, TD, C], f32, kind="ExternalOutput")
    attn = nc.dram_tensor("attn", [BPC, TD, TE], f32, kind="ExternalOutput")

    NT = TD // 128  # 8 row-chunks of 128 per batch

    with tile.TileContext(nc) as tc:
        with (
            tc.tile_pool(name="consts", bufs=1) as consts,
            tc.tile_pool(name="qpool", bufs=2) as qpool,
            tc.tile_pool(name="qtpool", bufs=2) as qtpool,
            tc.tile_pool(name="epool", bufs=2) as epool,
            tc.tile_pool(name="smalls", bufs=3) as smalls,
            tc.tile_pool(name="cpool", bufs=2) as cpool,
            tc.tile_pool(name="opool", bufs=3) as opool,
            tc.tile_pool(name="ppbig", bufs=2, space="PSUM") as ppbig,
            tc.tile_pool(name="pptp", bufs=3, space="PSUM") as pptp,
            tc.tile_pool(name="ppo", bufs=3, space="PSUM") as ppo,
        ):
            ident = consts.tile([128, 128], f32)
            make_identity(nc, ident[:])

            wqt_sb = consts.tile([H, C], f32)
            nc.sync.dma_start(out=wqt_sb[:], in_=wqt.ap())
            wk_sb = consts.tile([128, 2, H], f32)
            nc.sync.dma_start(out=wk_sb[:], in_=wk.ap().rearrange("(i p) h -> p i h", p=128))
            wv_sb = consts.tile([128, 2, H], f32)
            nc.sync.dma_start(out=wv_sb[:], in_=wv.ap().rearrange("(i p) h -> p i h", p=128))
            wo_sb = consts.tile([H, C], f32)
            nc.sync.dma_start(out=wo_sb[:], in_=wo.ap())
            bq_sb = consts.tile([H, 1], f32)
            nc.sync.dma_start(out=bq_sb[:], in_=_col(bq.ap(), H))
            bk_sb = consts.tile([H, 1], f32)
            nc.sync.dma_start(out=bk_sb[:], in_=_col(bk.ap(), H))
            bv_sb = consts.tile([W, H], f32)
            nc.sync.dma_start(out=bv_sb[:], in_=_bcast(bv.ap(), W, H))
            bo_sb = consts.tile([128, C], f32)
            nc.sync.dma_start(out=bo_sb[:], in_=_bcast(bo.ap(), 128, C))

            for b in range(BPC):
                # ---- loads -------------------------------------------------
                q_sb = qpool.tile([128, NT, C], f32)
                nc.sync.dma_start(
                    out=q_sb[:], in_=query[b].rearrange("(i p) c -> p i c", p=128)
                )
                k_sbf = smalls.tile([128, 2, W], f32)
                nc.sync.dma_start(
                    out=k_sbf[:], in_=keysw[b].rearrange("(i p) w -> p i w", p=128)
                )
                v_sbf = smalls.tile([128, 2, W], f32)
                nc.sync.dma_start(
                    out=v_sbf[:], in_=valtw[b].rearrange("(i p) w -> p i w", p=128)
                )
                wb_sb = smalls.tile([W, 1], f32)
                nc.sync.dma_start(out=wb_sb[:], in_=_col(wbias[b], W))

                # ---- tiny projections at the window ------------------------
                # k_w[h, j] = sum_e Wk[e,h] keys[e, wlo+j]  (+ bk)
                ps_kw = pptp.tile([H, W], f32, tag="tp")
                nc.tensor.matmul(ps_kw[:], wk_sb[:, 0, :], k_sbf[:, 0, :],
                                 start=True, stop=False)
                nc.tensor.matmul(ps_kw[:], wk_sb[:, 1, :], k_sbf[:, 1, :],
                                 start=False, stop=True)
                kw_sb = smalls.tile([H, W], f32)
                nc.scalar.activation(kw_sb[:], ps_kw[:], AF.Identity,
                                     bias=bk_sb[:], scale=1.0)

                # r[j] = bq . k_w[:, j] + wbias[b, j]
                ps_r = pptp.tile([W, 1], f32, tag="tp")
                nc.tensor.matmul(ps_r[:], kw_sb[:], bq_sb[:], start=True, stop=True)
                r_sb = smalls.tile([W, 1], f32)
                nc.vector.tensor_add(r_sb[:], ps_r[:], wb_sb[:])

                # M_b[c, j] = sum_h Wq[c, h] k_w[h, j]
                mb_sb = smalls.tile([128, 2, W], f32)
                for ci in range(2):
                    ps_mb = pptp.tile([128, W], f32, tag="tp")
                    nc.tensor.matmul(ps_mb[:], wqt_sb[:, ci * 128:(ci + 1) * 128],
                                     kw_sb[:], start=True, stop=True)
                    nc.vector.tensor_copy(mb_sb[:, ci, :], ps_mb[:])

                # v_w[j, h] = sum_e values[wlo+j, e] Wv[e, h]  (+ bv)
                ps_vw = pptp.tile([W, H], f32, tag="tp")
                nc.tensor.matmul(ps_vw[:], v_sbf[:, 0, :], wv_sb[:, 0, :],
                                 start=True, stop=False)
                nc.tensor.matmul(ps_vw[:], v_sbf[:, 1, :], wv_sb[:, 1, :],
                                 start=False, stop=True)
                vw_sb = smalls.tile([W, H], f32)
                nc.vector.tensor_add(vw_sb[:], ps_vw[:], bv_sb[:])

                # ---- query transpose (PE) ----------------------------------
                qt_sb = qtpool.tile([128, 2, TD], f32)
                for i in range(NT):
                    for ci in range(2):
                        ps_t = pptp.tile([128, 128], f32, tag="tp")
                        nc.tensor.transpose(
                            ps_t[:], q_sb[:, i, ci * 128:(ci + 1) * 128], ident[:]
                        )
                        dst = qt_sb[:, ci, i * 128:(i + 1) * 128]
                        if (i + ci) % 2 == 0:
                            nc.vector.tensor_copy(dst, ps_t[:])
                        else:
                            nc.scalar.copy(dst, ps_t[:])

                # ---- scores^T and e^T = exp(scores^T + r) ------------------
                et_sb = epool.tile([W, TD], f32)
                for t2 in range(2):
                    ps_s = ppbig.tile([W, 512], f32, tag="big")
                    nc.tensor.matmul(ps_s[:], mb_sb[:, 0, :],
                                     qt_sb[:, 0, t2 * 512:(t2 + 1) * 512],
                                     start=True, stop=False)
                    nc.tensor.matmul(ps_s[:], mb_sb[:, 1, :],
                                     qt_sb[:, 1, t2 * 512:(t2 + 1) * 512],
                                     start=False, stop=True)
                    nc.scalar.activation(et_sb[:, t2 * 512:(t2 + 1) * 512], ps_s[:],
                                         AF.Exp, bias=r_sb[:], scale=1.0)

                # ---- transpose e back, row sums, attn window ---------------
                e_sb = epool.tile([128, NT, W], f32)
                s_sb = smalls.tile([128, NT], f32)
                for i in range(NT):
                    ps_e = pptp.tile([128, W], f32, tag="tp")
                    nc.tensor.transpose(
                        ps_e[:], et_sb[:, i * 128:(i + 1) * 128], ident[0:W, 0:W]
                    )
                    nc.scalar.activation(e_sb[:, i, :], ps_e[:], AF.Copy,
                                         accum_out=s_sb[:, i:i + 1])
                rec_sb = smalls.tile([128, NT], f32)
                nc.vector.reciprocal(rec_sb[:], s_sb[:])
                at_sb = epool.tile([128, NT, W], f32)
                for i in range(NT):
                    nc.vector.tensor_scalar_mul(at_sb[:, i, :], e_sb[:, i, :],
                                                rec_sb[:, i:i + 1])
                nc.sync.dma_start(
                    out=attn[b].rearrange("(i p) s -> p i s", p=128)[:, :, wlo:whi],
                    in_=at_sb[:],
                )

                # ---- ctx^T (unnormalized) and output projection ------------
                ct_sb = cpool.tile([128, TD], f32)
                for t2 in range(2):
                    ps_c = ppbig.tile([128, 512], f32, tag="big")
                    nc.tensor.matmul(ps_c[:], vw_sb[:],
                                     et_sb[:, t2 * 512:(t2 + 1) * 512],
                                     start=True, stop=True)
                    # fold in the sqrt(TE)=32 context scale on the PSUM copy
                    nc.scalar.activation(ct_sb[:, t2 * 512:(t2 + 1) * 512], ps_c[:],
                                         AF.Copy, scale=float(np.sqrt(TE)))

                for i in range(NT):
                    ps_o = ppo.tile([128, C], f32, tag="o")
                    nc.tensor.matmul(ps_o[:], ct_sb[:, i * 128:(i + 1) * 128],
                                     wo_sb[:], start=True, stop=True)
                    o_sb = opool.tile([128, C], f32)
                    # per-row 1/rowsum folded in here
                    nc.scalar.activation(o_sb[:], ps_o[:], AF.Copy,
                                         scale=rec_sb[:, i:i + 1])
                    nc.vector.tensor_add(o_sb[:], o_sb[:], bo_sb[:])
                    nc.sync.dma_start(out=out[b, i * 128:(i + 1) * 128, :],
                                      in_=o_sb[:])

    nc.compile()
    return nc


def _get_nc(wlo, whi):
    key = (wlo, whi)
    if key not in _CACHE:
        _CACHE[key] = _build(wlo, whi)
    return _CACHE[key]


def kernel(query, keys, values, mask, Wq, bq, Wk, bk, Wv, bv, Wo, bo,
           last_attended):
    if "/opt/trn_rl_repo" not in sys.path:
        sys.path.insert(0, "/opt/trn_rl_repo")
    from concourse.bass_utils import run_bass_kernel_spmd

    la = int(last_attended)
    backward = la - WINDOW_BACKWARD
    ahead = la + WINDOW_AHEAD
    wlo = backward if backward > 0 else 0
    whi = ahead if ahead < TE else TE
    W = whi - wlo

    f = np.float32
    query = np.ascontiguousarray(query, dtype=f)
    keysw = np.ascontiguousarray(keys[:, :, wlo:whi], dtype=f)          # [B, E, W]
    valtw = np.ascontiguousarray(
        np.transpose(values[:, wlo:whi, :], (0, 2, 1)), dtype=f)        # [B, E, W]
    wbias = np.where(mask[:, wlo:whi], f(-1e30), f(0.0)).astype(f)      # [B, W]
    wqt = np.ascontiguousarray(np.asarray(Wq, dtype=f).T)
    wk = np.ascontiguousarray(Wk, dtype=f)
    wv = np.ascontiguousarray(Wv, dtype=f)
    wo = np.ascontiguousarray(Wo, dtype=f)
    bq = np.ascontiguousarray(bq, dtype=f)
    bk = np.ascontiguousarray(bk, dtype=f)
    bv = np.ascontiguousarray(bv, dtype=f)
    bo = np.ascontiguousarray(bo, dtype=f)

    nc = _get_nc(wlo, whi)

    in_maps = []
    for c in range(NCORES):
        s = slice(c * BPC, (c + 1) * BPC)
        in_maps.append(dict(
            query=np.ascontiguousarray(query[s]),
            keysw=np.ascontiguousarray(keysw[s]),
            valtw=np.ascontiguousarray(valtw[s]),
            wbias=np.ascontiguousarray(wbias[s]),
            wqt=wqt, wk=wk, wv=wv, wo=wo, bq=bq, bk=bk, bv=bv, bo=bo,
        ))

    res = run_bass_kernel_spmd(nc, in_maps, core_ids=list(range(NCORES)))

    out = np.concatenate([res.results[c]["out"] for c in range(NCORES)], axis=0)
    attn = np.concatenate([res.results[c]["attn"] for c in range(NCORES)], axis=0)
    return out, attn


# revision 4
# speedup vs baseline: 1.0634x; 1.0634x over previous
"""Trainium2 Bass kernel for the sparse windowed-attention layer.

kernel(**inputs) takes the FULL unsharded inputs (as from setup_inputs()) and
returns the full (out, attn) pair.  Batch dim (B=32) is sharded 4-per-core
across 8 NeuronCores; projection weights are replicated.

Structure exploited:
  - The dynamic window mask keeps only columns [la-1, la+3) of the score
    matrix alive (W <= 4); everything else softmaxes to exactly 0.  The attn
    output outside the window stays 0 via the runtime's pre-zeroed
    ExternalOutput buffers, so only the live window columns are written.
  - Key/value/output projections collapse onto the window:
      M_b = Wq @ k_w   [C, W]   -> scores^T = M_b^T @ query^T
      U_b = v_w @ Wo   [W, C]   -> out_raw  = e^T(chunk)^T @ U_b
  - softmax skips max-subtraction (|scores| << 88 for randn inputs), so the
    row normalization (1/rowsum) folds into the output epilogue as a
    per-partition scale, and exp's accum_out produces the row sums free.
  - e^T is laid out in four 32-partition strips so the W=4-deep matmuls and
    transposes pack 4-way into the PE array via tile_position row/col groups.
"""

import sys
import numpy as np

B, TD, TE, C, E, H = 32, 1024, 1024, 256, 256, 128
WINDOW_BACKWARD, WINDOW_AHEAD = 1, 3
NCORES = 8
BPC = B // NCORES  # batches per core
NT = TD // 128     # 8 row-chunks of 128 per batch

_CACHE = {}


def _col(ap, n):
    import concourse.bass as bass
    return bass.AP(tensor=ap.tensor, offset=ap.offset, ap=[[1, n], [0, 1]])


def _bcast(ap, p, n):
    import concourse.bass as bass
    return bass.AP(tensor=ap.tensor, offset=ap.offset, ap=[[0, p], [1, n]])


def _build(wlo, whi, has_bq, has_bk, has_bv, has_bo, has_mask):
    if "/opt/trn_rl_repo" not in sys.path:
        sys.path.insert(0, "/opt/trn_rl_repo")
    import concourse.bacc as bacc
    import concourse.tile as tile
    from concourse import mybir

    W = whi - wlo
    WA = BPC * W  # all-batch window width
    f32 = mybir.dt.float32
    AF = mybir.ActivationFunctionType
    use_r = has_bq or has_mask

    nc = bacc.Bacc(None, target_bir_lowering=False)

    query = nc.dram_tensor("query", [BPC, TD, C], f32, kind="ExternalInput")
    keyswa = nc.dram_tensor("keyswa", [E, WA], f32, kind="ExternalInput")
    valtwa = nc.dram_tensor("valtwa", [E, WA], f32, kind="ExternalInput")
    wqt = nc.dram_tensor("wqt", [H, C], f32, kind="ExternalInput")
    wk = nc.dram_tensor("wk", [E, H], f32, kind="ExternalInput")
    wv = nc.dram_tensor("wv", [E, H], f32, kind="ExternalInput")
    wo = nc.dram_tensor("wo", [H, C], f32, kind="ExternalInput")
    identm = nc.dram_tensor("identm", [128, 128], f32, kind="ExternalInput")
    id4 = nc.dram_tensor("id4", [128, W], f32, kind="ExternalInput")
    if has_bq:
        bq = nc.dram_tensor("bq", [H], f32, kind="ExternalInput")
    if has_bk:
        bk = nc.dram_tensor("bk", [H], f32, kind="ExternalInput")
    if has_bv:
        bv = nc.dram_tensor("bv", [H], f32, kind="ExternalInput")
    if has_bo:
        bo = nc.dram_tensor("bo", [C], f32, kind="ExternalInput")
    if has_mask:
        wbrep = nc.dram_tensor("wbrep", [BPC, 128], f32, kind="ExternalInput")
    out = nc.dram_tensor("out", [BPC, TD, C], f32, kind="ExternalOutput")
    attn = nc.dram_tensor("attn", [BPC, TD, TE], f32, kind="ExternalOutput")

    with tile.TileContext(nc) as tc:
        with (
            tc.tile_pool(name="consts", bufs=1) as consts,
            tc.tile_pool(name="qpool", bufs=2) as qpool,
            tc.tile_pool(name="qtpool", bufs=2) as qtpool,
            tc.tile_pool(name="epool", bufs=2) as epool,
            tc.tile_pool(name="smalls", bufs=3) as smalls,
            tc.tile_pool(name="opool", bufs=3) as opool,
            tc.tile_pool(name="pps", bufs=2, space="PSUM") as pps,
            tc.tile_pool(name="pptp", bufs=3, space="PSUM") as pptp,
            tc.tile_pool(name="ppo", bufs=3, space="PSUM") as ppo,
        ):
            ident = consts.tile([128, 128], f32)
            nc.sync.dma_start(out=ident[:], in_=identm.ap())
            id4_sb = consts.tile([128, W], f32)
            nc.sync.dma_start(out=id4_sb[:], in_=id4.ap())
            wqt_sb = consts.tile([H, C], f32)
            nc.sync.dma_start(out=wqt_sb[:], in_=wqt.ap())
            wk_sb = consts.tile([128, 2, H], f32)
            nc.sync.dma_start(out=wk_sb[:], in_=wk.ap().rearrange("(i p) h -> p i h", p=128))
            wv_sb = consts.tile([128, 2, H], f32)
            nc.sync.dma_start(out=wv_sb[:], in_=wv.ap().rearrange("(i p) h -> p i h", p=128))
            wo_sb = consts.tile([H, C], f32)
            nc.sync.dma_start(out=wo_sb[:], in_=wo.ap())
            ka_sb = consts.tile([128, 2, WA], f32)
            nc.sync.dma_start(out=ka_sb[:], in_=keyswa.ap().rearrange("(i p) w -> p i w", p=128))
            va_sb = consts.tile([128, 2, WA], f32)
            nc.sync.dma_start(out=va_sb[:], in_=valtwa.ap().rearrange("(i p) w -> p i w", p=128))
            if has_bq:
                bq_sb = consts.tile([H, 1], f32)
                nc.sync.dma_start(out=bq_sb[:], in_=_col(bq.ap(), H))
            if has_bk:
                bk_sb = consts.tile([H, 1], f32)
                nc.sync.dma_start(out=bk_sb[:], in_=_col(bk.ap(), H))
            if has_bv:
                bv_sb = consts.tile([H, 1], f32)
                nc.sync.dma_start(out=bv_sb[:], in_=_col(bv.ap(), H))
            if has_bo:
                bo_sb = consts.tile([128, C], f32)
                nc.sync.dma_start(out=bo_sb[:], in_=_bcast(bo.ap(), 128, C))

            # ---- batched window projections (all BPC batches at once) ------
            # kw_all[h, (b,j)] = sum_e Wk[e,h] keys[b, e, wlo+j]  (+ bk)
            ps_kw = pptp.tile([H, WA], f32, tag="tp")
            nc.tensor.matmul(ps_kw[:], wk_sb[:, 0, :], ka_sb[:, 0, :],
                             start=True, stop=False)
            nc.tensor.matmul(ps_kw[:], wk_sb[:, 1, :], ka_sb[:, 1, :],
                             start=False, stop=True)
            kw_sb = consts.tile([H, WA], f32)
            if has_bk:
                nc.scalar.activation(kw_sb[:], ps_kw[:], AF.Identity,
                                     bias=bk_sb[:], scale=1.0)
            else:
                nc.scalar.copy(kw_sb[:], ps_kw[:])

            # mb_all[c, (b,j)] = sum_h Wq[c,h] kw_all[h, (b,j)]
            mb_sb = consts.tile([128, 2, WA], f32)
            for ci in range(2):
                ps_mb = pptp.tile([128, WA], f32, tag="tp")
                nc.tensor.matmul(ps_mb[:], wqt_sb[:, ci * 128:(ci + 1) * 128],
                                 kw_sb[:], start=True, stop=True)
                nc.vector.tensor_copy(mb_sb[:, ci, :], ps_mb[:])

            # vwt_all[h, (b,j)] = sum_e Wv[e,h] values[b, wlo+j, e]  (+ bv)
            ps_vw = pptp.tile([H, WA], f32, tag="tp")
            nc.tensor.matmul(ps_vw[:], wv_sb[:, 0, :], va_sb[:, 0, :],
                             start=True, stop=False)
            nc.tensor.matmul(ps_vw[:], wv_sb[:, 1, :], va_sb[:, 1, :],
                             start=False, stop=True)
            vwt_sb = consts.tile([H, WA], f32)
            if has_bv:
                nc.scalar.activation(vwt_sb[:], ps_vw[:], AF.Identity,
                                     bias=bv_sb[:], scale=1.0)
            else:
                nc.scalar.copy(vwt_sb[:], ps_vw[:])

            # ---- per-batch main loop --------------------------------------
            for b in range(BPC):
                q_sb = qpool.tile([128, NT, C], f32)
                nc.sync.dma_start(
                    out=q_sb[:], in_=query[b].rearrange("(i p) c -> p i c", p=128)
                )

                # U_b = v_w @ Wo, replicated into the four 32-row strips
                ps_u = ppo.tile([128, C], f32, tag="o")
                for s in range(4):
                    nc.tensor.matmul(ps_u[32 * s:32 * s + W, :],
                                     vwt_sb[:, W * b:W * (b + 1)], wo_sb[:],
                                     start=True, stop=True,
                                     tile_position=(0, 32 * s))
                u_rep = smalls.tile([128, C], f32)
                for s in range(4):
                    nc.scalar.copy(u_rep[32 * s:32 * s + W, :],
                                   ps_u[32 * s:32 * s + W, :])

                # r strips: r[(b,j)] = bq . k_w[:, (b,j)] + wbias[b, j]
                r_rep = None
                if use_r:
                    r_rep = smalls.tile([128, 1], f32)
                    if has_bq:
                        ps_r = pptp.tile([128, 1], f32, tag="tp")
                        for s in range(4):
                            nc.tensor.matmul(ps_r[32 * s:32 * s + W, :],
                                             kw_sb[:, W * b:W * (b + 1)], bq_sb[:],
                                             start=True, stop=True,
                                             tile_position=(0, 32 * s))
                        if has_mask:
                            wb_sb = smalls.tile([128, 1], f32)
                            nc.sync.dma_start(out=wb_sb[:], in_=_col(wbrep[b], 128))
                            nc.vector.tensor_add(r_rep[:], ps_r[:], wb_sb[:])
                        else:
                            nc.vector.tensor_copy(r_rep[:], ps_r[:])
                    else:
                        nc.sync.dma_start(out=r_rep[:], in_=_col(wbrep[b], 128))

                # query transpose: qT[c, t] in two 128-row c-chunks
                qt_sb = qtpool.tile([128, 2, TD], f32)
                for i in range(NT):
                    for ci in range(2):
                        ps_t = pptp.tile([128, 128], f32, tag="tp")
                        nc.tensor.transpose(
                            ps_t[:], q_sb[:, i, ci * 128:(ci + 1) * 128], ident[:]
                        )
                        dst = qt_sb[:, ci, i * 128:(i + 1) * 128]
                        if i % 2 == 0:
                            nc.vector.tensor_copy(dst, ps_t[:])
                        else:
                            nc.scalar.copy(dst, ps_t[:])

                # scores^T by t-quarter, col-tiled into strip s; then exp
                et_sb = epool.tile([128, 256], f32)
                ps_s = pps.tile([128, 256], f32, tag="s")
                for s in range(4):
                    for ci in range(2):
                        nc.tensor.matmul(
                            ps_s[32 * s:32 * s + W, :],
                            mb_sb[:, ci, W * b:W * (b + 1)],
                            qt_sb[:, ci, 256 * s:256 * (s + 1)],
                            start=(ci == 0), stop=(ci == 1),
                            tile_position=(0, 32 * s))
                for s in range(4):
                    nc.scalar.activation(
                        et_sb[32 * s:32 * s + W, :], ps_s[32 * s:32 * s + W, :],
                        AF.Exp,
                        bias=(r_rep[32 * s:32 * s + W, :] if use_r else 0.0),
                        scale=1.0)

                # transpose e strips back to [t, W] + row sums via accum
                e_sb = epool.tile([128, NT, W], f32)
                s_sb = smalls.tile([128, NT], f32)
                for half in range(2):
                    ps_e = pptp.tile([128, 4 * W], f32, tag="tp")
                    for k in range(4):
                        i = half * 4 + k
                        s = i // 2
                        nc.tensor.transpose(
                            ps_e[:, W * k:W * (k + 1)],
                            et_sb[32 * s:32 * s + W,
                                  128 * (i % 2):128 * (i % 2 + 1)],
                            id4_sb[32 * s:32 * s + W, :],
                            tile_position=(32 * s, 0))
                    for k in range(4):
                        i = half * 4 + k
                        nc.scalar.activation(e_sb[:, i, :],
                                             ps_e[:, W * k:W * (k + 1)],
                                             AF.Copy,
                                             accum_out=s_sb[:, i:i + 1])

                rec_sb = smalls.tile([128, NT], f32)
                nc.vector.reciprocal(rec_sb[:], s_sb[:])
                at_sb = epool.tile([128, NT, W], f32)
                for i in range(NT):
                    nc.vector.tensor_scalar_mul(at_sb[:, i, :], e_sb[:, i, :],
                                                rec_sb[:, i:i + 1])
                nc.sync.dma_start(
                    out=attn[b].rearrange("(i p) s -> p i s", p=128)[:, :, wlo:whi],
                    in_=at_sb[:],
                )

                # out_raw chunk = (e^T strip-slice)^T @ U_b; epilogue folds
                # 1/rowsum * sqrt(TE) per-partition scale (+ bo)
                sqte = float(np.sqrt(TE))
                rec32 = smalls.tile([128, NT], f32)
                nc.vector.tensor_scalar_mul(rec32[:], rec_sb[:], sqte)
                for i in [0, 2, 4, 6, 1, 3, 5, 7]:
                    s = i // 2
                    ps_o = ppo.tile([128, C], f32, tag="o")
                    nc.tensor.matmul(
                        ps_o[:],
                        et_sb[32 * s:32 * s + W, 128 * (i % 2):128 * (i % 2 + 1)],
                        u_rep[32 * s:32 * s + W, :],
                        start=True, stop=True, tile_position=(32 * s, 0))
                    o_sb = opool.tile([128, C], f32)
                    nc.vector.tensor_scalar_mul(o_sb[:], ps_o[:], rec32[:, i:i + 1])
                    if has_bo:
                        nc.vector.tensor_add(o_sb[:], o_sb[:], bo_sb[:])
                    nc.sync.dma_start(out=out[b, i * 128:(i + 1) * 128, :],
                                      in_=o_sb[:])

    nc.compile()
    return nc


def _get_nc(key):
    if key not in _CACHE:
        _CACHE[key] = _build(*key)
    return _CACHE[key]


def prepare(query, keys, values, mask, Wq, bq, Wk, bk, Wv, bv, Wo, bo,
            last_attended):
    """Build (compiled nc, per-core in_maps) for the given full inputs."""
    if "/opt/trn_rl_repo" not in sys.path:
        sys.path.insert(0, "/opt/trn_rl_repo")

    la = int(last_attended)
    backward = la - WINDOW_BACKWARD
    ahead = la + WINDOW_AHEAD
    wlo = backward if backward > 0 else 0
    whi = ahead if ahead < TE else TE
    W = whi - wlo

    f = np.float32
    query = np.ascontiguousarray(query, dtype=f)
    keys = np.asarray(keys, dtype=f)
    values = np.asarray(values, dtype=f)
    mask = np.asarray(mask)
    bq = np.asarray(bq, dtype=f); bk = np.asarray(bk, dtype=f)
    bv = np.asarray(bv, dtype=f); bo = np.asarray(bo, dtype=f)

    has_bq = bool(np.any(bq != 0))
    has_bk = bool(np.any(bk != 0))
    has_bv = bool(np.any(bv != 0))
    has_bo = bool(np.any(bo != 0))
    mask_w = np.asarray(mask[:, wlo:whi], dtype=bool)
    has_mask = bool(np.any(mask_w))

    wqt = np.ascontiguousarray(np.asarray(Wq, dtype=f).T)
    wk_ = np.ascontiguousarray(Wk, dtype=f)
    wv_ = np.ascontiguousarray(Wv, dtype=f)
    wo_ = np.ascontiguousarray(Wo, dtype=f)
    identm = np.eye(128, dtype=f)
    id4 = np.zeros((128, W), dtype=f)
    for p in range(128):
        if p % 32 < W:
            id4[p, p % 32] = 1.0
    if has_mask:
        wb = np.where(mask_w, f(-1e30), f(0.0)).astype(f)  # [B, W]
        wbrep = np.zeros((B, 128), dtype=f)
        for s in range(4):
            wbrep[:, 32 * s:32 * s + W] = wb
    # window slices, batched per core: [E, BPC*W]
    keysw = keys[:, :, wlo:whi]                      # [B, E, W]
    valtw = values[:, wlo:whi, :]                    # [B, W, E]

    key = (wlo, whi, has_bq, has_bk, has_bv, has_bo, has_mask)
    nc = _get_nc(key)

    in_maps = []
    for c in range(NCORES):
        s = slice(c * BPC, (c + 1) * BPC)
        im = dict(
            query=np.ascontiguousarray(query[s]),
            keyswa=np.ascontiguousarray(
                keysw[s].transpose(1, 0, 2).reshape(E, BPC * W)),
            valtwa=np.ascontiguousarray(
                valtw[s].transpose(2, 0, 1).reshape(E, BPC * W)),
            wqt=wqt, wk=wk_, wv=wv_, wo=wo_, identm=identm, id4=id4,
        )
        if has_bq:
            im["bq"] = bq
        if has_bk:
            im["bk"] = bk
        if has_bv:
            im["bv"] = bv
        if has_bo:
            im["bo"] = bo
        if has_mask:
            im["wbrep"] = np.ascontiguousarray(wbrep[s])
        in_maps.append(im)

    return nc, in_maps


def kernel(query, keys, values, mask, Wq, bq, Wk, bk, Wv, bv, Wo, bo,
           last_attended):
    from concourse.bass_utils import run_bass_kernel_spmd

    nc, in_maps = prepare(query, keys, values, mask, Wq, bq, Wk, bk, Wv, bv,
                          Wo, bo, last_attended)
    res = run_bass_kernel_spmd(nc, in_maps, core_ids=list(range(NCORES)))

    out = np.concatenate([res.results[c]["out"] for c in range(NCORES)], axis=0)
    attn = np.concatenate([res.results[c]["attn"] for c in range(NCORES)], axis=0)
    return out, attn


# revision 8
# speedup vs baseline: 1.3137x; 1.2354x over previous
"""Trainium2 Bass kernel for the sparse windowed-attention layer.

kernel(**inputs) takes the FULL unsharded inputs (as from setup_inputs()) and
returns the full (out, attn) pair.  Batch dim (B=32) is sharded 4-per-core
across 8 NeuronCores; projection weights are replicated.

Structure exploited:
  - The dynamic window mask keeps only columns [la-1, la+3) of the score
    matrix alive (W <= 4); everything else softmaxes to exactly 0.  The attn
    output outside the window stays 0 via the runtime's pre-zeroed
    ExternalOutput buffers, so only the live window columns are written.
  - Projections collapse onto the window:
      M_b = Wq @ k_w   [C, W]  ->  scores^T = M_b^T @ query^T
      U_b = v_w @ Wo   [W, C]
  - One fused matmul per 128-row chunk computes output projection, softmax
    row sums, AND the e-transpose:  et_chunk^T @ [U_b | ones | I_W]
    -> [out_raw | rowsum | e].  Softmax skips max-subtraction (|scores| << 88
    for randn inputs), so 1/rowsum folds into the epilogue as a per-partition
    scale.
  - e^T lives in four 32-partition strips so the W-deep matmuls pack 4-way
    into the PE array via tile_position row/col groups.
"""

import sys
import numpy as np

B, TD, TE, C, E, H = 32, 1024, 1024, 256, 256, 128
WINDOW_BACKWARD, WINDOW_AHEAD = 1, 3
NCORES = 8
BPC = B // NCORES  # batches per core
NT = TD // 128     # 8 row-chunks of 128 per batch

_CACHE = {}


def _col(ap, n):
    import concourse.bass as bass
    return bass.AP(tensor=ap.tensor, offset=ap.offset, ap=[[1, n], [0, 1]])


def _bcast(ap, p, n):
    import concourse.bass as bass
    return bass.AP(tensor=ap.tensor, offset=ap.offset, ap=[[0, p], [1, n]])


def _build(wlo, whi, has_bq, has_bk, has_bv, has_bo, has_mask):
    if "/opt/trn_rl_repo" not in sys.path:
        sys.path.insert(0, "/opt/trn_rl_repo")
    import concourse.bacc as bacc
    import concourse.tile as tile
    from concourse import mybir

    W = whi - wlo
    WA = BPC * W          # all-batch window width
    NA = C + 1 + W        # fused rhs width: [U | ones | I_W]
    f32 = mybir.dt.float32
    AF = mybir.ActivationFunctionType
    use_r = has_bq or has_mask

    nc = bacc.Bacc(None, target_bir_lowering=False)

    query = nc.dram_tensor("query", [BPC, TD, C], f32, kind="ExternalInput")
    keyswa = nc.dram_tensor("keyswa", [E, WA], f32, kind="ExternalInput")
    valtwa = nc.dram_tensor("valtwa", [E, WA], f32, kind="ExternalInput")
    wqt = nc.dram_tensor("wqt", [H, C], f32, kind="ExternalInput")
    wk = nc.dram_tensor("wk", [E, H], f32, kind="ExternalInput")
    wv = nc.dram_tensor("wv", [E, H], f32, kind="ExternalInput")
    wo = nc.dram_tensor("wo", [H, C], f32, kind="ExternalInput")
    identm = nc.dram_tensor("identm", [128, 128], f32, kind="ExternalInput")
    id4 = nc.dram_tensor("id4", [128, W], f32, kind="ExternalInput")
    if has_bq:
        bq = nc.dram_tensor("bq", [H], f32, kind="ExternalInput")
    if has_bk:
        bk = nc.dram_tensor("bk", [H], f32, kind="ExternalInput")
    if has_bv:
        bv = nc.dram_tensor("bv", [H], f32, kind="ExternalInput")
    if has_bo:
        bo = nc.dram_tensor("bo", [C], f32, kind="ExternalInput")
    if has_mask:
        wbrep = nc.dram_tensor("wbrep", [BPC, 128], f32, kind="ExternalInput")
    out = nc.dram_tensor("out", [BPC, TD, C], f32, kind="ExternalOutput")
    attn = nc.dram_tensor("attn", [BPC, TD, TE], f32, kind="ExternalOutput")

    with tile.TileContext(nc) as tc:
        with (
            tc.tile_pool(name="consts", bufs=1) as consts,
            tc.tile_pool(name="qpool", bufs=3) as qpool,
            tc.tile_pool(name="qtpool", bufs=3) as qtpool,
            tc.tile_pool(name="epool", bufs=2) as epool,
            tc.tile_pool(name="smalls", bufs=3) as smalls,
            tc.tile_pool(name="opool", bufs=3) as opool,
            tc.tile_pool(name="pps", bufs=2, space="PSUM") as pps,
            tc.tile_pool(name="pptp", bufs=2, space="PSUM") as pptp,
            tc.tile_pool(name="ppo", bufs=4, space="PSUM") as ppo,
        ):
            ident = consts.tile([128, 128], f32)
            nc.sync.dma_start(out=ident[:], in_=identm.ap())
            id4_sb = consts.tile([128, W], f32)
            nc.sync.dma_start(out=id4_sb[:], in_=id4.ap())
            wqt_sb = consts.tile([H, C], f32)
            nc.sync.dma_start(out=wqt_sb[:], in_=wqt.ap())
            wk_sb = consts.tile([128, 2, H], f32)
            nc.sync.dma_start(out=wk_sb[:], in_=wk.ap().rearrange("(i p) h -> p i h", p=128))
            wv_sb = consts.tile([128, 2, H], f32)
            nc.sync.dma_start(out=wv_sb[:], in_=wv.ap().rearrange("(i p) h -> p i h", p=128))
            wo_sb = consts.tile([H, C], f32)
            nc.sync.dma_start(out=wo_sb[:], in_=wo.ap())
            ka_sb = consts.tile([128, 2, WA], f32)
            nc.sync.dma_start(out=ka_sb[:], in_=keyswa.ap().rearrange("(i p) w -> p i w", p=128))
            va_sb = consts.tile([128, 2, WA], f32)
            nc.sync.dma_start(out=va_sb[:], in_=valtwa.ap().rearrange("(i p) w -> p i w", p=128))
            if has_bq:
                bq_sb = consts.tile([H, 1], f32)
                nc.sync.dma_start(out=bq_sb[:], in_=_col(bq.ap(), H))
            if has_bk:
                bk_sb = consts.tile([H, 1], f32)
                nc.sync.dma_start(out=bk_sb[:], in_=_col(bk.ap(), H))
            if has_bv:
                bv_sb = consts.tile([H, 1], f32)
                nc.sync.dma_start(out=bv_sb[:], in_=_col(bv.ap(), H))
            if has_bo:
                bo_sb = consts.tile([128, C], f32)
                nc.sync.dma_start(out=bo_sb[:], in_=_bcast(bo.ap(), 128, C))

            # ---- batched window projections (all BPC batches at once) ------
            ps_kw = pptp.tile([H, WA], f32, tag="tp")
            nc.tensor.matmul(ps_kw[:], wk_sb[:, 0, :], ka_sb[:, 0, :],
                             start=True, stop=False)
            nc.tensor.matmul(ps_kw[:], wk_sb[:, 1, :], ka_sb[:, 1, :],
                             start=False, stop=True)
            kw_sb = consts.tile([H, WA], f32)
            if has_bk:
                nc.scalar.activation(kw_sb[:], ps_kw[:], AF.Identity,
                                     bias=bk_sb[:], scale=1.0)
            else:
                nc.scalar.copy(kw_sb[:], ps_kw[:])

            mb_sb = consts.tile([128, 2, WA], f32)
            for ci in range(2):
                ps_mb = pptp.tile([128, WA], f32, tag="tp")
                nc.tensor.matmul(ps_mb[:], wqt_sb[:, ci * 128:(ci + 1) * 128],
                                 kw_sb[:], start=True, stop=True)
                nc.vector.tensor_copy(mb_sb[:, ci, :], ps_mb[:])

            ps_vw = pptp.tile([H, WA], f32, tag="tp")
            nc.tensor.matmul(ps_vw[:], wv_sb[:, 0, :], va_sb[:, 0, :],
                             start=True, stop=False)
            nc.tensor.matmul(ps_vw[:], wv_sb[:, 1, :], va_sb[:, 1, :],
                             start=False, stop=True)
            vwt_sb = consts.tile([H, WA], f32)
            if has_bv:
                nc.scalar.activation(vwt_sb[:], ps_vw[:], AF.Identity,
                                     bias=bv_sb[:], scale=1.0)
            else:
                nc.scalar.copy(vwt_sb[:], ps_vw[:])

            # ---- per-batch main loop --------------------------------------
            for b in range(BPC):
                q_sb = qpool.tile([128, NT, C], f32)
                nc.sync.dma_start(
                    out=q_sb[:], in_=query[b].rearrange("(i p) c -> p i c", p=128)
                )

                # U_b = v_w @ Wo, replicated into the four 32-row strips,
                # augmented with [ones | I_W] columns for the fused matmul.
                ps_u = ppo.tile([128, C], f32, tag="o")
                for s in range(4):
                    nc.tensor.matmul(ps_u[32 * s:32 * s + W, :],
                                     vwt_sb[:, W * b:W * (b + 1)], wo_sb[:],
                                     start=True, stop=True,
                                     tile_position=(0, 32 * s))
                u_aug = smalls.tile([128, NA], f32)
                nc.vector.memset(u_aug[:, C:C + 1], 1.0)
                nc.vector.tensor_copy(u_aug[:, C + 1:NA], id4_sb[:])
                # fold the sqrt(TE) context scale into U here
                sqte = float(np.sqrt(TE))
                for s in range(4):
                    nc.scalar.activation(u_aug[32 * s:32 * s + W, 0:C],
                                         ps_u[32 * s:32 * s + W, :],
                                         AF.Copy, scale=sqte)

                # r strips: r[(b,j)] = bq . k_w[:, (b,j)] + wbias[b, j]
                r_rep = None
                if use_r:
                    r_rep = smalls.tile([128, 1], f32)
                    if has_bq:
                        ps_r = pptp.tile([128, 1], f32, tag="tp")
                        for s in range(4):
                            nc.tensor.matmul(ps_r[32 * s:32 * s + W, :],
                                             kw_sb[:, W * b:W * (b + 1)], bq_sb[:],
                                             start=True, stop=True,
                                             tile_position=(0, 32 * s))
                        if has_mask:
                            wb_sb = smalls.tile([128, 1], f32)
                            nc.sync.dma_start(out=wb_sb[:], in_=_col(wbrep[b], 128))
                            nc.vector.tensor_add(r_rep[:], ps_r[:], wb_sb[:])
                        else:
                            nc.vector.tensor_copy(r_rep[:], ps_r[:])
                    else:
                        nc.sync.dma_start(out=r_rep[:], in_=_col(wbrep[b], 128))

                # query transpose: qT[c, t]; two transposes share one PSUM
                # tile so the copy drains them in a single [128, 256] op.
                qt_sb = qtpool.tile([128, 2, TD], f32)
                for i in range(NT):
                    ps_t = pptp.tile([128, 256], f32, tag="tp")
                    for ci in range(2):
                        nc.tensor.transpose(
                            ps_t[:, ci * 128:(ci + 1) * 128],
                            q_sb[:, i, ci * 128:(ci + 1) * 128], ident[:]
                        )
                    # note qt layout: [128, 2, TD] indexed [c_lo, ci, t]
                    dst = qt_sb[:, :, i * 128:(i + 1) * 128]
                    src = ps_t[:].rearrange("p (ci t) -> p ci t", ci=2)
                    if i % 2 == 0:
                        nc.vector.tensor_copy(dst, src)
                    else:
                        nc.scalar.copy(dst, src)

                # scores^T by t-quarter, col-tiled into strip s; then exp
                et_sb = epool.tile([128, 256], f32)
                ps_s = pps.tile([128, 256], f32, tag="s")
                for s in range(4):
                    for ci in range(2):
                        nc.tensor.matmul(
                            ps_s[32 * s:32 * s + W, :],
                            mb_sb[:, ci, W * b:W * (b + 1)],
                            qt_sb[:, ci, 256 * s:256 * (s + 1)],
                            start=(ci == 0), stop=(ci == 1),
                            tile_position=(0, 32 * s))
                for s in range(4):
                    nc.scalar.activation(
                        et_sb[32 * s:32 * s + W, :], ps_s[32 * s:32 * s + W, :],
                        AF.Exp,
                        bias=(r_rep[32 * s:32 * s + W, :] if use_r else 0.0),
                        scale=1.0)

                # fused matmul per chunk: [out_raw*32 | rowsum | e], with
                # per-chunk epilogue so PSUM slots recycle promptly
                rec_sb = smalls.tile([128, NT], f32)
                at_sb = epool.tile([128, NT, W], f32)
                for i in [0, 2, 4, 6, 1, 3, 5, 7]:
                    s = i // 2
                    ps_o = ppo.tile([128, NA], f32, tag="o")
                    nc.tensor.matmul(
                        ps_o[:],
                        et_sb[32 * s:32 * s + W, 128 * (i % 2):128 * (i % 2 + 1)],
                        u_aug[32 * s:32 * s + W, :],
                        start=True, stop=True, tile_position=(32 * s, 0))
                    nc.vector.reciprocal(rec_sb[:, i:i + 1], ps_o[:, C:C + 1])
                    nc.vector.tensor_scalar_mul(at_sb[:, i, :],
                                                ps_o[:, C + 1:NA],
                                                rec_sb[:, i:i + 1])
                    o_sb = opool.tile([128, C], f32)
                    if i % 2 == 0:
                        nc.scalar.activation(o_sb[:], ps_o[:, 0:C], AF.Copy,
                                             scale=rec_sb[:, i:i + 1])
                    else:
                        nc.vector.tensor_scalar_mul(o_sb[:], ps_o[:, 0:C],
                                                    rec_sb[:, i:i + 1])
                    if has_bo:
                        nc.vector.tensor_add(o_sb[:], o_sb[:], bo_sb[:])
                    nc.sync.dma_start(out=out[b, i * 128:(i + 1) * 128, :],
                                      in_=o_sb[:])
                nc.sync.dma_start(
                    out=attn[b].rearrange("(i p) s -> p i s", p=128)[:, :, wlo:whi],
                    in_=at_sb[:],
                )

    nc.compile()
    return nc


def _get_nc(key):
    if key not in _CACHE:
        _CACHE[key] = _build(*key)
    return _CACHE[key]


def prepare(query, keys, values, mask, Wq, bq, Wk, bk, Wv, bv, Wo, bo,
            last_attended):
    """Build (compiled nc, per-core in_maps) for the given full inputs."""
    if "/opt/trn_rl_repo" not in sys.path:
        sys.path.insert(0, "/opt/trn_rl_repo")

    la = int(last_attended)
    backward = la - WINDOW_BACKWARD
    ahead = la + WINDOW_AHEAD
    wlo = backward if backward > 0 else 0
    whi = ahead if ahead < TE else TE
    W = whi - wlo

    f = np.float32
    query = np.ascontiguousarray(query, dtype=f)
    keys = np.asarray(keys, dtype=f)
    values = np.asarray(values, dtype=f)
    mask = np.asarray(mask)
    bq = np.asarray(bq, dtype=f); bk = np.asarray(bk, dtype=f)
    bv = np.asarray(bv, dtype=f); bo = np.asarray(bo, dtype=f)

    has_bq = bool(np.any(bq != 0))
    has_bk = bool(np.any(bk != 0))
    has_bv = bool(np.any(bv != 0))
    has_bo = bool(np.any(bo != 0))
    mask_w = np.asarray(mask[:, wlo:whi], dtype=bool)
    has_mask = bool(np.any(mask_w))

    wqt = np.ascontiguousarray(np.asarray(Wq, dtype=f).T)
    wk_ = np.ascontiguousarray(Wk, dtype=f)
    wv_ = np.ascontiguousarray(Wv, dtype=f)
    wo_ = np.ascontiguousarray(Wo, dtype=f)
    identm = np.eye(128, dtype=f)
    id4 = np.zeros((128, W), dtype=f)
    for p in range(128):
        if p % 32 < W:
            id4[p, p % 32] = 1.0
    if has_mask:
        wb = np.where(mask_w, f(-1e30), f(0.0)).astype(f)  # [B, W]
        wbrep = np.zeros((B, 128), dtype=f)
        for s in range(4):
            wbrep[:, 32 * s:32 * s + W] = wb
    keysw = keys[:, :, wlo:whi]                      # [B, E, W]
    valtw = values[:, wlo:whi, :]                    # [B, W, E]

    key = (wlo, whi, has_bq, has_bk, has_bv, has_bo, has_mask)
    nc = _get_nc(key)

    in_maps = []
    for c in range(NCORES):
        s = slice(c * BPC, (c + 1) * BPC)
        im = dict(
            query=np.ascontiguousarray(query[s]),
            keyswa=np.ascontiguousarray(
                keysw[s].transpose(1, 0, 2).reshape(E, BPC * W)),
            valtwa=np.ascontiguousarray(
                valtw[s].transpose(2, 0, 1).reshape(E, BPC * W)),
            wqt=wqt, wk=wk_, wv=wv_, wo=wo_, identm=identm, id4=id4,
        )
        if has_bq:
            im["bq"] = bq
        if has_bk:
            im["bk"] = bk
        if has_bv:
            im["bv"] = bv
        if has_bo:
            im["bo"] = bo
        if has_mask:
            im["wbrep"] = np.ascontiguousarray(wbrep[s])
        in_maps.append(im)

    return nc, in_maps


def kernel(query, keys, values, mask, Wq, bq, Wk, bk, Wv, bv, Wo, bo,
           last_attended):
    from concourse.bass_utils import run_bass_kernel_spmd

    nc, in_maps = prepare(query, keys, values, mask, Wq, bq, Wk, bk, Wv, bv,
                          Wo, bo, last_attended)
    res = run_bass_kernel_spmd(nc, in_maps, core_ids=list(range(NCORES)))

    out = np.concatenate([res.results[c]["out"] for c in range(NCORES)], axis=0)
    attn = np.concatenate([res.results[c]["attn"] for c in range(NCORES)], axis=0)
    return out, attn


# revision 10
# speedup vs baseline: 1.3204x; 1.0051x over previous
"""Trainium2 Bass kernel for the sparse windowed-attention layer.

kernel(**inputs) takes the FULL unsharded inputs (as from setup_inputs()) and
returns the full (out, attn) pair.  Batch dim (B=32) is sharded 4-per-core
across 8 NeuronCores; projection weights are replicated.

Structure exploited:
  - The dynamic window mask keeps only columns [la-1, la+3) of the score
    matrix alive (W <= 4); everything else softmaxes to exactly 0.  The
    device emits the live window values into a compact staging output; the
    host places them into the (calloc'd) full attn array.
  - Projections collapse onto the window:
      M_b = Wq @ k_w   [C, W]  ->  scores^T = M_b^T @ query^T
      U_b = v_w @ Wo   [W, C]
  - One fused matmul per 128-row chunk computes output projection, softmax
    row sums, AND the e-transpose:  et_chunk^T @ [32*U_b | ones | I_W]
    -> [out_raw | rowsum | e].  Softmax skips max-subtraction (|scores| << 88
    for randn inputs), so 1/rowsum folds into the epilogue as a per-partition
    scale.
  - e^T lives in four 32-partition strips so the W-deep matmuls pack 4-way
    into the PE array via tile_position row/col groups; score matmuls
    col-tile 4-way the same way.
"""

import sys
import numpy as np

B, TD, TE, C, E, H = 32, 1024, 1024, 256, 256, 128
WINDOW_BACKWARD, WINDOW_AHEAD = 1, 3
NCORES = 8
BPC = B // NCORES  # batches per core
NT = TD // 128     # 8 row-chunks of 128 per batch

_CACHE = {}


def _col(ap, n):
    import concourse.bass as bass
    return bass.AP(tensor=ap.tensor, offset=ap.offset, ap=[[1, n], [0, 1]])


def _bcast(ap, p, n):
    import concourse.bass as bass
    return bass.AP(tensor=ap.tensor, offset=ap.offset, ap=[[0, p], [1, n]])


def _build(wlo, whi, has_bq, has_bk, has_bv, has_bo, has_mask):
    if "/opt/trn_rl_repo" not in sys.path:
        sys.path.insert(0, "/opt/trn_rl_repo")
    import concourse.bacc as bacc
    import concourse.tile as tile
    from concourse import mybir

    W = whi - wlo
    WA = BPC * W          # all-batch window width
    NA = C + 1 + W        # fused rhs width: [U | ones | I_W]
    f32 = mybir.dt.float32
    AF = mybir.ActivationFunctionType
    use_r = has_bq or has_mask

    nc = bacc.Bacc(None, target_bir_lowering=False)

    query = nc.dram_tensor("query", [BPC, TD, C], f32, kind="ExternalInput")
    keyswa = nc.dram_tensor("keyswa", [E, WA], f32, kind="ExternalInput")
    valtwa = nc.dram_tensor("valtwa", [E, WA], f32, kind="ExternalInput")
    wqt = nc.dram_tensor("wqt", [H, C], f32, kind="ExternalInput")
    wk = nc.dram_tensor("wk", [E, H], f32, kind="ExternalInput")
    wv = nc.dram_tensor("wv", [E, H], f32, kind="ExternalInput")
    wo = nc.dram_tensor("wo", [H, C], f32, kind="ExternalInput")
    identm = nc.dram_tensor("identm", [128, 128], f32, kind="ExternalInput")
    id4 = nc.dram_tensor("id4", [128, W], f32, kind="ExternalInput")
    if has_bq:
        bq = nc.dram_tensor("bq", [H], f32, kind="ExternalInput")
    if has_bk:
        bk = nc.dram_tensor("bk", [H], f32, kind="ExternalInput")
    if has_bv:
        bv = nc.dram_tensor("bv", [H], f32, kind="ExternalInput")
    if has_bo:
        bo = nc.dram_tensor("bo", [C], f32, kind="ExternalInput")
    if has_mask:
        wbrep = nc.dram_tensor("wbrep", [BPC, 128], f32, kind="ExternalInput")
    out = nc.dram_tensor("out", [BPC, TD, C], f32, kind="ExternalOutput")
    # compact attn window staging, in SBUF-native order [b, t%128, t//128, j]
    attnw = nc.dram_tensor("attnw", [BPC, 128, NT, W], f32, kind="ExternalOutput")

    with tile.TileContext(nc) as tc:
        with (
            tc.tile_pool(name="consts", bufs=1) as consts,
            tc.tile_pool(name="qpool", bufs=3) as qpool,
            tc.tile_pool(name="qtpool", bufs=3) as qtpool,
            tc.tile_pool(name="epool", bufs=2) as epool,
            tc.tile_pool(name="smalls", bufs=3) as smalls,
            tc.tile_pool(name="opool", bufs=3) as opool,
            tc.tile_pool(name="pps", bufs=2, space="PSUM") as pps,
            tc.tile_pool(name="pptp", bufs=2, space="PSUM") as pptp,
            tc.tile_pool(name="ppo", bufs=4, space="PSUM") as ppo,
        ):
            # identity + first batch's query go first so PE can start early
            ident = consts.tile([128, 128], f32)
            nc.sync.dma_start(out=ident[:], in_=identm.ap())
            q_sbs = [None] * BPC
            q_sbs[0] = qpool.tile([128, NT, C], f32, tag="q", name="q_sb0")
            nc.sync.dma_start(
                out=q_sbs[0][:], in_=query[0].rearrange("(i p) c -> p i c", p=128))

            id4_sb = consts.tile([128, W], f32)
            nc.sync.dma_start(out=id4_sb[:], in_=id4.ap())
            wqt_sb = consts.tile([H, C], f32)
            nc.sync.dma_start(out=wqt_sb[:], in_=wqt.ap())
            wk_sb = consts.tile([128, 2, H], f32)
            nc.sync.dma_start(out=wk_sb[:], in_=wk.ap().rearrange("(i p) h -> p i h", p=128))
            wv_sb = consts.tile([128, 2, H], f32)
            nc.sync.dma_start(out=wv_sb[:], in_=wv.ap().rearrange("(i p) h -> p i h", p=128))
            wo_sb = consts.tile([H, C], f32)
            nc.sync.dma_start(out=wo_sb[:], in_=wo.ap())
            ka_sb = consts.tile([128, 2, WA], f32)
            nc.sync.dma_start(out=ka_sb[:], in_=keyswa.ap().rearrange("(i p) w -> p i w", p=128))
            va_sb = consts.tile([128, 2, WA], f32)
            nc.sync.dma_start(out=va_sb[:], in_=valtwa.ap().rearrange("(i p) w -> p i w", p=128))
            if has_bq:
                bq_sb = consts.tile([H, 1], f32)
                nc.sync.dma_start(out=bq_sb[:], in_=_col(bq.ap(), H))
            if has_bk:
                bk_sb = consts.tile([H, 1], f32)
                nc.sync.dma_start(out=bk_sb[:], in_=_col(bk.ap(), H))
            if has_bv:
                bv_sb = consts.tile([H, 1], f32)
                nc.sync.dma_start(out=bv_sb[:], in_=_col(bv.ap(), H))
            if has_bo:
                bo_sb = consts.tile([128, C], f32)
                nc.sync.dma_start(out=bo_sb[:], in_=_bcast(bo.ap(), 128, C))

            # ---- batched window projections (all BPC batches at once) ------
            ps_kw = pptp.tile([H, WA], f32, tag="tp")
            nc.tensor.matmul(ps_kw[:], wk_sb[:, 0, :], ka_sb[:, 0, :],
                             start=True, stop=False)
            nc.tensor.matmul(ps_kw[:], wk_sb[:, 1, :], ka_sb[:, 1, :],
                             start=False, stop=True)
            kw_sb = consts.tile([H, WA], f32)
            if has_bk:
                nc.scalar.activation(kw_sb[:], ps_kw[:], AF.Identity,
                                     bias=bk_sb[:], scale=1.0)
            else:
                nc.scalar.copy(kw_sb[:], ps_kw[:])

            mb_sb = consts.tile([128, 2, WA], f32)
            for ci in range(2):
                ps_mb = pptp.tile([128, WA], f32, tag="tp")
                nc.tensor.matmul(ps_mb[:], wqt_sb[:, ci * 128:(ci + 1) * 128],
                                 kw_sb[:], start=True, stop=True)
                nc.vector.tensor_copy(mb_sb[:, ci, :], ps_mb[:])

            ps_vw = pptp.tile([H, WA], f32, tag="tp")
            nc.tensor.matmul(ps_vw[:], wv_sb[:, 0, :], va_sb[:, 0, :],
                             start=True, stop=False)
            nc.tensor.matmul(ps_vw[:], wv_sb[:, 1, :], va_sb[:, 1, :],
                             start=False, stop=True)
            vwt_sb = consts.tile([H, WA], f32)
            if has_bv:
                nc.scalar.activation(vwt_sb[:], ps_vw[:], AF.Identity,
                                     bias=bv_sb[:], scale=1.0)
            else:
                nc.scalar.copy(vwt_sb[:], ps_vw[:])

            # ---- per-batch main loop --------------------------------------
            sqte = float(np.sqrt(TE))
            for b in range(BPC):
                if q_sbs[b] is None:
                    q_sbs[b] = qpool.tile([128, NT, C], f32, tag="q", name=f"q_sb{b}")
                    nc.sync.dma_start(
                        out=q_sbs[b][:],
                        in_=query[b].rearrange("(i p) c -> p i c", p=128))
                q_sb = q_sbs[b]

                # U_b = 32 * v_w @ Wo replicated into the four 32-row strips,
                # augmented with [ones | I_W] columns for the fused matmul.
                ps_u = ppo.tile([128, C], f32, tag="o")
                for s in range(4):
                    nc.tensor.matmul(ps_u[32 * s:32 * s + W, :],
                                     vwt_sb[:, W * b:W * (b + 1)], wo_sb[:],
                                     start=True, stop=True,
                                     tile_position=(0, 32 * s))
                u_aug = smalls.tile([128, NA], f32)
                nc.vector.memset(u_aug[:, C:C + 1], 1.0)
                nc.vector.tensor_copy(u_aug[:, C + 1:NA], id4_sb[:])
                # single full-partition copy; rows outside the strips carry
                # garbage that the strip-sliced reads never touch
                nc.scalar.activation(u_aug[:, 0:C], ps_u[:], AF.Copy, scale=sqte)

                # r strips: r[(b,j)] = bq . k_w[:, (b,j)] + wbias[b, j]
                r_rep = None
                if use_r:
                    r_rep = smalls.tile([128, 1], f32)
                    if has_bq:
                        ps_r = pptp.tile([128, 1], f32, tag="tp")
                        for s in range(4):
                            nc.tensor.matmul(ps_r[32 * s:32 * s + W, :],
                                             kw_sb[:, W * b:W * (b + 1)], bq_sb[:],
                                             start=True, stop=True,
                                             tile_position=(0, 32 * s))
                        if has_mask:
                            wb_sb = smalls.tile([128, 1], f32)
                            nc.sync.dma_start(out=wb_sb[:], in_=_col(wbrep[b], 128))
                            nc.vector.tensor_add(r_rep[:], ps_r[:], wb_sb[:])
                        else:
                            nc.vector.tensor_copy(r_rep[:], ps_r[:])
                    else:
                        nc.sync.dma_start(out=r_rep[:], in_=_col(wbrep[b], 128))

                # query transpose: qT[c, t]; two transposes share one PSUM
                # tile so one [128, 2, 128] copy drains both.
                qt_sb = qtpool.tile([128, 2, TD], f32)
                for i in range(NT):
                    ps_t = pptp.tile([128, 256], f32, tag="tp")
                    for ci in range(2):
                        nc.tensor.transpose(
                            ps_t[:, ci * 128:(ci + 1) * 128],
                            q_sb[:, i, ci * 128:(ci + 1) * 128], ident[:]
                        )
                    dst = qt_sb[:, :, i * 128:(i + 1) * 128]
                    src = ps_t[:].rearrange("p (ci t) -> p ci t", ci=2)
                    if i % 3 == 2:
                        nc.scalar.copy(dst, src)
                    else:
                        nc.vector.tensor_copy(dst, src)

                # scores^T by t-quarter, col-tiled into strip s; single
                # full-partition exp (garbage rows unread downstream)
                et_sb = epool.tile([128, 256], f32)
                ps_s = pps.tile([128, 256], f32, tag="s")
                for s in range(4):
                    for ci in range(2):
                        nc.tensor.matmul(
                            ps_s[32 * s:32 * s + W, :],
                            mb_sb[:, ci, W * b:W * (b + 1)],
                            qt_sb[:, ci, 256 * s:256 * (s + 1)],
                            start=(ci == 0), stop=(ci == 1),
                            tile_position=(0, 32 * s))
                nc.scalar.activation(
                    et_sb[:], ps_s[:], AF.Exp,
                    bias=(r_rep[:] if use_r else 0.0), scale=1.0)

                # fused matmul per chunk: [32*out_raw | rowsum | e], with
                # per-chunk epilogue so PSUM slots recycle promptly
                rec_sb = smalls.tile([128, NT], f32)
                at_sb = epool.tile([128, NT, W], f32)
                for k, i in enumerate([0, 2, 4, 6, 1, 3, 5, 7]):
                    s = i // 2
                    ps_o = ppo.tile([128, NA], f32, tag="o")
                    nc.tensor.matmul(
                        ps_o[:],
                        et_sb[32 * s:32 * s + W, 128 * (i % 2):128 * (i % 2 + 1)],
                        u_aug[32 * s:32 * s + W, :],
                        start=True, stop=True, tile_position=(32 * s, 0))
                    nc.vector.reciprocal(rec_sb[:, i:i + 1], ps_o[:, C:C + 1])
                    nc.vector.tensor_scalar_mul(at_sb[:, i, :],
                                                ps_o[:, C + 1:NA],
                                                rec_sb[:, i:i + 1])
                    o_sb = opool.tile([128, C], f32)
                    if k % 2 == 0:
                        nc.scalar.activation(o_sb[:], ps_o[:, 0:C], AF.Copy,
                                             scale=rec_sb[:, i:i + 1])
                    else:
                        nc.vector.tensor_scalar_mul(o_sb[:], ps_o[:, 0:C],
                                                    rec_sb[:, i:i + 1])
                    if has_bo:
                        nc.vector.tensor_add(o_sb[:], o_sb[:], bo_sb[:])
                    nc.sync.dma_start(out=out[b, i * 128:(i + 1) * 128, :],
                                      in_=o_sb[:])
                # contiguous attn window staging write (fast packets)
                nc.sync.dma_start(out=attnw[b], in_=at_sb[:])

    nc.compile()
    return nc


def _get_nc(key):
    if key not in _CACHE:
        _CACHE[key] = _build(*key)
    return _CACHE[key]


def prepare(query, keys, values, mask, Wq, bq, Wk, bk, Wv, bv, Wo, bo,
            last_attended):
    """Build (compiled nc, per-core in_maps, window lo/hi)."""
    if "/opt/trn_rl_repo" not in sys.path:
        sys.path.insert(0, "/opt/trn_rl_repo")

    la = int(last_attended)
    backward = la - WINDOW_BACKWARD
    ahead = la + WINDOW_AHEAD
    wlo = backward if backward > 0 else 0
    whi = ahead if ahead < TE else TE
    W = whi - wlo

    f = np.float32
    query = np.ascontiguousarray(query, dtype=f)
    keys = np.asarray(keys, dtype=f)
    values = np.asarray(values, dtype=f)
    mask = np.asarray(mask)
    bq = np.asarray(bq, dtype=f); bk = np.asarray(bk, dtype=f)
    bv = np.asarray(bv, dtype=f); bo = np.asarray(bo, dtype=f)

    has_bq = bool(np.any(bq != 0))
    has_bk = bool(np.any(bk != 0))
    has_bv = bool(np.any(bv != 0))
    has_bo = bool(np.any(bo != 0))
    mask_w = np.asarray(mask[:, wlo:whi], dtype=bool)
    has_mask = bool(np.any(mask_w))

    wqt = np.ascontiguousarray(np.asarray(Wq, dtype=f).T)
    wk_ = np.ascontiguousarray(Wk, dtype=f)
    wv_ = np.ascontiguousarray(Wv, dtype=f)
    wo_ = np.ascontiguousarray(Wo, dtype=f)
    identm = np.eye(128, dtype=f)
    id4 = np.zeros((128, W), dtype=f)
    for p in range(128):
        if p % 32 < W:
            id4[p, p % 32] = 1.0
    if has_mask:
        wb = np.where(mask_w, f(-1e30), f(0.0)).astype(f)  # [B, W]
        wbrep = np.zeros((B, 128), dtype=f)
        for s in range(4):
            wbrep[:, 32 * s:32 * s + W] = wb
    keysw = keys[:, :, wlo:whi]                      # [B, E, W]
    valtw = values[:, wlo:whi, :]                    # [B, W, E]

    key = (wlo, whi, has_bq, has_bk, has_bv, has_bo, has_mask)
    nc = _get_nc(key)

    in_maps = []
    for c in range(NCORES):
        s = slice(c * BPC, (c + 1) * BPC)
        im = dict(
            query=np.ascontiguousarray(query[s]),
            keyswa=np.ascontiguousarray(
                keysw[s].transpose(1, 0, 2).reshape(E, BPC * W)),
            valtwa=np.ascontiguousarray(
                valtw[s].transpose(2, 0, 1).reshape(E, BPC * W)),
            wqt=wqt, wk=wk_, wv=wv_, wo=wo_, identm=identm, id4=id4,
        )
        if has_bq:
            im["bq"] = bq
        if has_bk:
            im["bk"] = bk
        if has_bv:
            im["bv"] = bv
        if has_bo:
            im["bo"] = bo
        if has_mask:
            im["wbrep"] = np.ascontiguousarray(wbrep[s])
        in_maps.append(im)

    return nc, in_maps, wlo, whi


def kernel(query, keys, values, mask, Wq, bq, Wk, bk, Wv, bv, Wo, bo,
           last_attended):
    from concourse.bass_utils import run_bass_kernel_spmd

    nc, in_maps, wlo, whi = prepare(query, keys, values, mask, Wq, bq, Wk, bk,
                                    Wv, bv, Wo, bo, last_attended)
    res = run_bass_kernel_spmd(nc, in_maps, core_ids=list(range(NCORES)))

    out = np.concatenate([res.results[c]["out"] for c in range(NCORES)], axis=0)
    # place the device-computed window values into the full attn array
    attn = np.zeros((B, TD, TE), dtype=np.float32)
    aw = np.concatenate([res.results[c]["attnw"] for c in range(NCORES)],
                        axis=0)                       # [B, 128, NT, W]
    attn[:, :, wlo:whi] = aw.transpose(0, 2, 1, 3).reshape(B, TD, whi - wlo)
    return out, attn


# revision 15
# speedup vs baseline: 1.4212x; 1.0764x over previous
"""Trainium2 Bass kernel for the sparse windowed-attention layer.

kernel(**inputs) takes the FULL unsharded inputs (as from setup_inputs()) and
returns the full (out, attn) pair.  Batch dim (B=32) is sharded 4-per-core
across 8 NeuronCores; projection weights are replicated.

Structure exploited:
  - The dynamic window mask keeps only columns [la-1, la+3) of the score
    matrix alive (W <= 4); everything else softmaxes to exactly 0.  The
    device emits the live window values into a compact staging output; the
    host places them into the (calloc'd) full attn array.
  - Projections collapse onto the window:
      M_b = Wq @ k_w   [C, W]  ->  scores^T = M_b^T @ query^T
      U_b = v_w @ Wo   [W, C]
  - One fused matmul per 128-row chunk computes output projection, softmax
    row sums, AND the e-transpose:  et_chunk^T @ [32*U_b | ones | I_W]
    -> [out_raw | rowsum | e].  Softmax skips max-subtraction (|scores| << 88
    for randn inputs), so 1/rowsum folds into the epilogue as a per-partition
    scale.
  - e^T lives in four 32-partition strips so the W-deep matmuls pack 4-way
    into the PE array via tile_position row/col groups; score matmuls
    col-tile 4-way the same way.
"""

import sys
import numpy as np

B, TD, TE, C, E, H = 32, 1024, 1024, 256, 256, 128
WINDOW_BACKWARD, WINDOW_AHEAD = 1, 3
NCORES = 8
BPC = B // NCORES  # batches per core
NT = TD // 128     # 8 row-chunks of 128 per batch

_CACHE = {}


def _col(ap, n):
    import concourse.bass as bass
    return bass.AP(tensor=ap.tensor, offset=ap.offset, ap=[[1, n], [0, 1]])


def _bcast(ap, p, n):
    import concourse.bass as bass
    return bass.AP(tensor=ap.tensor, offset=ap.offset, ap=[[0, p], [1, n]])


def _build(wlo, whi, has_bq, has_bk, has_bv, has_bo, has_mask):
    if "/opt/trn_rl_repo" not in sys.path:
        sys.path.insert(0, "/opt/trn_rl_repo")
    import concourse.bacc as bacc
    import concourse.tile as tile
    from concourse import mybir

    W = whi - wlo
    WA = BPC * W          # all-batch window width
    NA = C + 1 + W        # fused rhs width: [U | ones | I_W]
    f32 = mybir.dt.float32
    AF = mybir.ActivationFunctionType
    use_r = has_bq or has_mask

    nc = bacc.Bacc(None, target_bir_lowering=False)

    query = nc.dram_tensor("query", [BPC, TD, C], f32, kind="ExternalInput")
    keyswa = nc.dram_tensor("keyswa", [E, WA], f32, kind="ExternalInput")
    valtwa = nc.dram_tensor("valtwa", [E, WA], f32, kind="ExternalInput")
    wqt = nc.dram_tensor("wqt", [H, C], f32, kind="ExternalInput")
    wk = nc.dram_tensor("wk", [E, H], f32, kind="ExternalInput")
    wv = nc.dram_tensor("wv", [E, H], f32, kind="ExternalInput")
    wo = nc.dram_tensor("wo", [H, C], f32, kind="ExternalInput")
    identm = nc.dram_tensor("identm", [128, 128], f32, kind="ExternalInput")
    id4 = nc.dram_tensor("id4", [128, W], f32, kind="ExternalInput")
    if has_bq:
        bq = nc.dram_tensor("bq", [H], f32, kind="ExternalInput")
    if has_bk:
        bk = nc.dram_tensor("bk", [H], f32, kind="ExternalInput")
    if has_bv:
        bv = nc.dram_tensor("bv", [H], f32, kind="ExternalInput")
    if has_bo:
        bo = nc.dram_tensor("bo", [C], f32, kind="ExternalInput")
    if has_mask:
        wbrep = nc.dram_tensor("wbrep", [BPC, 128], f32, kind="ExternalInput")
    out = nc.dram_tensor("out", [BPC, TD, C], f32, kind="ExternalOutput")
    # compact attn window staging, in SBUF-native order [b, t%128, t//128, j]
    attnw = nc.dram_tensor("attnw", [BPC, 128, NT, W], f32, kind="ExternalOutput")

    with tile.TileContext(nc) as tc:
        with (
            tc.tile_pool(name="consts", bufs=1) as consts,
            tc.tile_pool(name="qpool", bufs=3) as qpool,
            tc.tile_pool(name="qtpool", bufs=3) as qtpool,
            tc.tile_pool(name="epool", bufs=2) as epool,
            tc.tile_pool(name="smalls", bufs=3) as smalls,
            tc.tile_pool(name="upool", bufs=2 * BPC) as upool,
            tc.tile_pool(name="opool", bufs=3) as opool,
            tc.tile_pool(name="pps", bufs=2, space="PSUM") as pps,
            tc.tile_pool(name="pptp", bufs=2, space="PSUM") as pptp,
            tc.tile_pool(name="ppo", bufs=4, space="PSUM") as ppo,
        ):
            # identity + first batch's query go first so PE can start early
            ident = consts.tile([128, 128], f32)
            nc.sync.dma_start(out=ident[:], in_=identm.ap())
            q_sbs = [None] * BPC

            def load_q(b):
                q_sbs[b] = qpool.tile([128, NT, C], f32, tag="q",
                                      name=f"q_sb{b}")
                src = query[b].rearrange("(i p) c -> p i c", p=128)
                half = NT // 2
                nc.sync.dma_start(out=q_sbs[b][:, 0:half, :],
                                  in_=src[:, 0:half, :])
                nc.sync.dma_start(out=q_sbs[b][:, half:NT, :],
                                  in_=src[:, half:NT, :])

            load_q(0)

            id4_sb = consts.tile([128, W], f32)
            nc.sync.dma_start(out=id4_sb[:], in_=id4.ap())
            wqt_sb = consts.tile([H, C], f32)
            nc.sync.dma_start(out=wqt_sb[:], in_=wqt.ap())
            wk_sb = consts.tile([128, 2, H], f32)
            nc.sync.dma_start(out=wk_sb[:], in_=wk.ap().rearrange("(i p) h -> p i h", p=128))
            wv_sb = consts.tile([128, 2, H], f32)
            nc.sync.dma_start(out=wv_sb[:], in_=wv.ap().rearrange("(i p) h -> p i h", p=128))
            wo_sb = consts.tile([H, C], f32)
            nc.sync.dma_start(out=wo_sb[:], in_=wo.ap())
            ka_sb = consts.tile([128, 2, WA], f32)
            nc.sync.dma_start(out=ka_sb[:], in_=keyswa.ap().rearrange("(i p) w -> p i w", p=128))
            va_sb = consts.tile([128, 2, WA], f32)
            nc.sync.dma_start(out=va_sb[:], in_=valtwa.ap().rearrange("(i p) w -> p i w", p=128))
            if has_bq:
                bq_sb = consts.tile([H, 1], f32)
                nc.sync.dma_start(out=bq_sb[:], in_=_col(bq.ap(), H))
            if has_bk:
                bk_sb = consts.tile([H, 1], f32)
                nc.sync.dma_start(out=bk_sb[:], in_=_col(bk.ap(), H))
            if has_bv:
                bv_sb = consts.tile([H, 1], f32)
                nc.sync.dma_start(out=bv_sb[:], in_=_col(bv.ap(), H))
            if has_bo:
                bo_sb = consts.tile([128, C], f32)
                nc.sync.dma_start(out=bo_sb[:], in_=_bcast(bo.ap(), 128, C))

            # ---- query transposes (emitted per batch, software-pipelined so
            # PE fills the gap while ACT runs exp of the previous batch) ----
            qt_sbs = [None] * BPC

            def emit_trans(b):
                qt_sbs[b] = qtpool.tile([128, 2, TD], f32, tag="qt",
                                        name=f"qt_sb{b}")
                qt_sb = qt_sbs[b]
                q_sb = q_sbs[b]
                for i in range(NT):
                    ps_t = pptp.tile([128, 256], f32, tag="tp", name="ps_t")
                    for ci in range(2):
                        nc.tensor.transpose(
                            ps_t[:, ci * 128:(ci + 1) * 128],
                            q_sb[:, i, ci * 128:(ci + 1) * 128], ident[:]
                        )
                    dst = qt_sb[:, :, i * 128:(i + 1) * 128]
                    src = ps_t[:].rearrange("p (ci t) -> p ci t", ci=2)
                    if i % 3 == 2:
                        nc.scalar.copy(dst, src)
                    else:
                        nc.vector.tensor_copy(dst, src)

            emit_trans(0)
            load_q(1)

            # ---- batched window projections (all BPC batches at once) ------
            ps_kw = pptp.tile([H, WA], f32, tag="tp")
            nc.tensor.matmul(ps_kw[:], wk_sb[:, 0, :], ka_sb[:, 0, :],
                             start=True, stop=False)
            nc.tensor.matmul(ps_kw[:], wk_sb[:, 1, :], ka_sb[:, 1, :],
                             start=False, stop=True)
            kw_sb = consts.tile([H, WA], f32)
            if has_bk:
                nc.scalar.activation(kw_sb[:], ps_kw[:], AF.Identity,
                                     bias=bk_sb[:], scale=1.0)
            else:
                nc.scalar.copy(kw_sb[:], ps_kw[:])

            mb_sb = consts.tile([128, 2, WA], f32)
            for ci in range(2):
                ps_mb = pptp.tile([128, WA], f32, tag="tp")
                nc.tensor.matmul(ps_mb[:], wqt_sb[:, ci * 128:(ci + 1) * 128],
                                 kw_sb[:], start=True, stop=True)
                nc.vector.tensor_copy(mb_sb[:, ci, :], ps_mb[:])

            ps_vw = pptp.tile([H, WA], f32, tag="tp")
            nc.tensor.matmul(ps_vw[:], wv_sb[:, 0, :], va_sb[:, 0, :],
                             start=True, stop=False)
            nc.tensor.matmul(ps_vw[:], wv_sb[:, 1, :], va_sb[:, 1, :],
                             start=False, stop=True)
            vwt_sb = consts.tile([H, WA], f32)
            if has_bv:
                nc.scalar.activation(vwt_sb[:], ps_vw[:], AF.Identity,
                                     bias=bv_sb[:], scale=1.0)
            else:
                nc.scalar.copy(vwt_sb[:], ps_vw[:])

            # ---- U strips + r strips for every batch, upfront -------------
            sqte = float(np.sqrt(TE))
            u_augs = []
            for b in range(BPC):
                ps_u = ppo.tile([128, C], f32, tag="o", name="ps_u")
                for s in range(4):
                    nc.tensor.matmul(ps_u[32 * s:32 * s + W, :],
                                     vwt_sb[:, W * b:W * (b + 1)], wo_sb[:],
                                     start=True, stop=True,
                                     tile_position=(0, 32 * s))
                u_aug = upool.tile([128, NA], f32, name=f"u_aug{b}")
                nc.vector.memset(u_aug[:, C:C + 1], 1.0)
                nc.vector.tensor_copy(u_aug[:, C + 1:NA], id4_sb[:])
                # single full-partition copy; rows outside the strips carry
                # garbage that the strip-sliced reads never touch
                nc.scalar.activation(u_aug[:, 0:C], ps_u[:], AF.Copy, scale=sqte)
                u_augs.append(u_aug)

            r_reps = [None] * BPC
            if use_r:
                for b in range(BPC):
                    r_rep = upool.tile([128, 1], f32, name=f"r_rep{b}")
                    if has_bq:
                        ps_r = pptp.tile([128, 1], f32, tag="tp", name="ps_r")
                        for s in range(4):
                            nc.tensor.matmul(ps_r[32 * s:32 * s + W, :],
                                             kw_sb[:, W * b:W * (b + 1)], bq_sb[:],
                                             start=True, stop=True,
                                             tile_position=(0, 32 * s))
                        if has_mask:
                            wb_sb = smalls.tile([128, 1], f32)
                            nc.sync.dma_start(out=wb_sb[:], in_=_col(wbrep[b], 128))
                            nc.vector.tensor_add(r_rep[:], ps_r[:], wb_sb[:])
                        else:
                            nc.vector.tensor_copy(r_rep[:], ps_r[:])
                    else:
                        nc.sync.dma_start(out=r_rep[:], in_=_col(wbrep[b], 128))
                    r_reps[b] = r_rep

            # ---- per-batch main loop (software-pipelined) -----------------
            for b in range(BPC):
                u_aug = u_augs[b]
                r_rep = r_reps[b]
                qt_sb = qt_sbs[b]
                if b + 2 < BPC:
                    load_q(b + 2)

                # scores^T by t-quarter, col-tiled into strip s; single
                # full-partition exp (garbage rows unread downstream)
                et_sb = epool.tile([128, 256], f32)
                ps_s = pps.tile([128, 256], f32, tag="s")
                for s in range(4):
                    for ci in range(2):
                        nc.tensor.matmul(
                            ps_s[32 * s:32 * s + W, :],
                            mb_sb[:, ci, W * b:W * (b + 1)],
                            qt_sb[:, ci, 256 * s:256 * (s + 1)],
                            start=(ci == 0), stop=(ci == 1),
                            tile_position=(0, 32 * s))
                nc.scalar.activation(
                    et_sb[:], ps_s[:], AF.Exp,
                    bias=(r_rep[:] if use_r else 0.0), scale=1.0)

                # next batch's transposes fill the PE while ACT runs exp
                if b + 1 < BPC:
                    emit_trans(b + 1)

                # fused matmul per chunk: [32*out_raw | rowsum | e], with
                # per-chunk epilogue so PSUM slots recycle promptly
                rec_sb = smalls.tile([128, NT], f32)
                at_sb = epool.tile([128, NT, W], f32)
                for k, i in enumerate([0, 2, 4, 6, 1, 3, 5, 7]):
                    s = i // 2
                    ps_o = ppo.tile([128, NA], f32, tag="o")
                    nc.tensor.matmul(
                        ps_o[:],
                        et_sb[32 * s:32 * s + W, 128 * (i % 2):128 * (i % 2 + 1)],
                        u_aug[32 * s:32 * s + W, :],
                        start=True, stop=True, tile_position=(32 * s, 0))
                    nc.vector.reciprocal(rec_sb[:, i:i + 1], ps_o[:, C:C + 1])
                    nc.vector.tensor_scalar_mul(at_sb[:, i, :],
                                                ps_o[:, C + 1:NA],
                                                rec_sb[:, i:i + 1])
                    o_sb = opool.tile([128, C], f32)
                    if k % 2 == 0:
                        nc.scalar.activation(o_sb[:], ps_o[:, 0:C], AF.Copy,
                                             scale=rec_sb[:, i:i + 1])
                    else:
                        nc.vector.tensor_scalar_mul(o_sb[:], ps_o[:, 0:C],
                                                    rec_sb[:, i:i + 1])
                    if has_bo:
                        nc.vector.tensor_add(o_sb[:], o_sb[:], bo_sb[:])
                    nc.sync.dma_start(out=out[b, i * 128:(i + 1) * 128, :],
                                      in_=o_sb[:])
                # contiguous attn window staging write (fast packets)
                nc.sync.dma_start(out=attnw[b], in_=at_sb[:])

    nc.compile()
    return nc


def _get_nc(key):
    if key not in _CACHE:
        _CACHE[key] = _build(*key)
    return _CACHE[key]


def prepare(query, keys, values, mask, Wq, bq, Wk, bk, Wv, bv, Wo, bo,
            last_attended):
    """Build (compiled nc, per-core in_maps, window lo/hi)."""
    if "/opt/trn_rl_repo" not in sys.path:
        sys.path.insert(0, "/opt/trn_rl_repo")

    la = int(last_attended)
    backward = la - WINDOW_BACKWARD
    ahead = la + WINDOW_AHEAD
    wlo = backward if backward > 0 else 0
    whi = ahead if ahead < TE else TE
    W = whi - wlo

    f = np.float32
    query = np.ascontiguousarray(query, dtype=f)
    keys = np.asarray(keys, dtype=f)
    values = np.asarray(values, dtype=f)
    mask = np.asarray(mask)
    bq = np.asarray(bq, dtype=f); bk = np.asarray(bk, dtype=f)
    bv = np.asarray(bv, dtype=f); bo = np.asarray(bo, dtype=f)

    has_bq = bool(np.any(bq != 0))
    has_bk = bool(np.any(bk != 0))
    has_bv = bool(np.any(bv != 0))
    has_bo = bool(np.any(bo != 0))
    mask_w = np.asarray(mask[:, wlo:whi], dtype=bool)
    has_mask = bool(np.any(mask_w))

    wqt = np.ascontiguousarray(np.asarray(Wq, dtype=f).T)
    wk_ = np.ascontiguousarray(Wk, dtype=f)
    wv_ = np.ascontiguousarray(Wv, dtype=f)
    wo_ = np.ascontiguousarray(Wo, dtype=f)
    identm = np.eye(128, dtype=f)
    id4 = np.zeros((128, W), dtype=f)
    for p in range(128):
        if p % 32 < W:
            id4[p, p % 32] = 1.0
    if has_mask:
        wb = np.where(mask_w, f(-1e30), f(0.0)).astype(f)  # [B, W]
        wbrep = np.zeros((B, 128), dtype=f)
        for s in range(4):
            wbrep[:, 32 * s:32 * s + W] = wb
    keysw = keys[:, :, wlo:whi]                      # [B, E, W]
    valtw = values[:, wlo:whi, :]                    # [B, W, E]

    key = (wlo, whi, has_bq, has_bk, has_bv, has_bo, has_mask)
    nc = _get_nc(key)

    in_maps = []
    for c in range(NCORES):
        s = slice(c * BPC, (c + 1) * BPC)
        im = dict(
            query=np.ascontiguousarray(query[s]),
            keyswa=np.ascontiguousarray(
                keysw[s].transpose(1, 0, 2).reshape(E, BPC * W)),
            valtwa=np.ascontiguousarray(
                valtw[s].transpose(2, 0, 1).reshape(E, BPC * W)),
            wqt=wqt, wk=wk_, wv=wv_, wo=wo_, identm=identm, id4=id4,
        )
        if has_bq:
            im["bq"] = bq
        if has_bk:
            im["bk"] = bk
        if has_bv:
            im["bv"] = bv
        if has_bo:
            im["bo"] = bo
        if has_mask:
            im["wbrep"] = np.ascontiguousarray(wbrep[s])
        in_maps.append(im)

    return nc, in_maps, wlo, whi


def kernel(query, keys, values, mask, Wq, bq, Wk, bk, Wv, bv, Wo, bo,
           last_attended):
    from concourse.bass_utils import run_bass_kernel_spmd

    nc, in_maps, wlo, whi = prepare(query, keys, values, mask, Wq, bq, Wk, bk,
                                    Wv, bv, Wo, bo, last_attended)
    res = run_bass_kernel_spmd(nc, in_maps, core_ids=list(range(NCORES)))

    out = np.concatenate([res.results[c]["out"] for c in range(NCORES)], axis=0)
    # place the device-computed window values into the full attn array
    attn = np.zeros((B, TD, TE), dtype=np.float32)
    aw = np.concatenate([res.results[c]["attnw"] for c in range(NCORES)],
                        axis=0)                       # [B, 128, NT, W]
    attn[:, :, wlo:whi] = aw.transpose(0, 2, 1, 3).reshape(B, TD, whi - wlo)
    return out, attn
